# revision 29
# baseline (speedup 1.0000x reference)
"""Trainium2 Bass kernel for nn_Decoder (attention-LSTM decoder recurrence).

Math (per batch b, T=128 steps, M=P=64):
    UH = H @ U_d.T                                  (hoisted)
    repeat t = 0..T-2:
        q = [h; c]                                  (128,)
        e = tanh(UH + W_d @ q)                      (T, M)
        s = exp(v_d . e)                            (T,)   softmax numerator
        num = sum_t s_t * (H w~[1:] + w~b)_t        per-batch scalar
        den = sum_t s_t
        y~  = w~0 * dec_t + num / den
        LSTM(y~, h, c) -> h, c                      (i,f,g,o gates)
    final: attend once more; out = [h, ctx]

Sharding: data-parallel over batch. B=32 over 8 cores -> 4 batches/core.

v4 design: the 4 local batches form 2 independent chains of 2 batches,
software-pipelined half a step apart.  All per-step element-wise work is
expressed as single-free-column ops ([P,1] slices), which the engines
process at negligible cost; the only wide op per chain-step is the
attention tanh ([128, T], on Act).  The LSTM tail runs entirely on Act
as fused func(in*scale + bias) ops (scale/bias per-partition vectors,
f32 scale), eliminating the DVE round-trips; sigmoid is folded to tanh
via sigmoid(z) = 0.5 tanh(0.5 z) + 0.5 with the 0.5s packed into the
weights (state kept as 2h, 2c in f16).  Per chain-step the serial chain
is PE(qp) > Act(e) > PE(lg) > Act(exp) > PE(den/num) > DVE(recip) >
PE(wih rank-1) > Act(gates+state), and the two chains' engine streams
are interleaved so neither blocks the other (emission order below).
Preamble: 6 packed DMAs + 4 f16 UH matmuls (UH stays PSUM-resident).
"""

import contextlib

import numpy as np

B, T, M, P = 32, 128, 64, 64
NCORES = 8
BL = B // NCORES          # batches per core = 4
NG = 2                    # pipelines (groups) per core
GB = BL // NG             # batches per group = 2

_STATE = {}


def _build_nc(pt=1112, w0=4000):
    import concourse.bacc as bacc
    import concourse.tile as tile
    from concourse import mybir

    f32 = mybir.dt.float32
    f32r = mybir.dt.float32r
    f16 = mybir.dt.float16
    AF = mybir.ActivationFunctionType
    OP = mybir.AluOpType

    nc = bacc.Bacc()

    # ---- packed inputs (few big DMAs) ----
    # blobA [128, 455] f16: wdT2 | udT | whh64 (rows 0:64) ; v2|ones|hw16
    # blobB [2, 1024] f16: dec2 (0:512) | wdec2 (512:768; wih = row 1)
    # htp [64, 4T] f16, haugp [T, 4*65] f32, st0 [128, BL] f16
    blobA = nc.declare_dram_parameter("blobA", [2 * M, 455], f16, isOutput=False)
    blobB = nc.declare_dram_parameter("blobB", [2, 1024], f16, isOutput=False)
    htp_d = nc.declare_dram_parameter("htp", [M, BL * T], f16, isOutput=False)
    haugp_d = nc.declare_dram_parameter(
        "haugp", [T, BL * (M + 1)], f32, isOutput=False)
    st0 = nc.declare_dram_parameter("st0", [2 * P, BL], f16, isOutput=False)
    # ---- outputs ----
    oh = nc.declare_dram_parameter("oh", [P, BL], f16, isOutput=True)
    octx = nc.declare_dram_parameter("octx", [M + 1, BL], f32, isOutput=True)

    def mm(out, lhsT, rhs, **kw):
        nc.tensor.matmul(out, lhsT.bitcast(f32r), rhs.bitcast(f32r), **kw)

    with tile.TileContext(nc) as tc:
        with contextlib.ExitStack() as ctx:
            consts = ctx.enter_context(tc.tile_pool(name="consts", bufs=1))
            state = ctx.enter_context(tc.tile_pool(name="state", bufs=1))
            loop_sb = ctx.enter_context(tc.tile_pool(name="loop_sb", bufs=3))
            # bufs=1 pool whose tiles are shared by BOTH groups: the WAW/WAR
            # chains through the single buffer totally order group 0's ops
            # before group 1's within each stage (same-engine, zero cost),
            # preventing the scheduler from interleaving the two groups'
            # stage blocks (which strands group 0's later ops behind group
            # 1's not-yet-ready ones, +200ns/step).
            pin_sb = ctx.enter_context(tc.tile_pool(name="pin_sb", bufs=1))
            loop_ps = ctx.enter_context(
                tc.tile_pool(name="loop_ps", bufs=1, space="PSUM")
            )
            uh_pool = ctx.enter_context(
                tc.tile_pool(name="uh_ps", bufs=1, space="PSUM")
            )

            # ---------------- preamble: 6 packed DMAs ----------------
            blobA_sb = consts.tile([2 * M, 455], f16, tag="BLOBA")
            nc.sync.dma_start(out=blobA_sb, in_=blobA[:])
            htp = consts.tile([M, BL * T], f16, tag="HTP")
            nc.gpsimd.dma_start(out=htp, in_=htp_d[:])
            haugp = consts.tile([T, BL * (M + 1)], f32r, tag="HAUGP")
            nc.sync.dma_start(out=haugp, in_=haugp_d[:].bitcast(f32r))
            blobB_sb = consts.tile([2, 1024], f16, tag="BLOBB")
            nc.gpsimd.dma_start(out=blobB_sb, in_=blobB[:])

            wdT2_sb = blobA_sb[0:P, 0:2 * M]
            udT_sb = blobA_sb[0:M, 2 * M:2 * M + M]
            whh_sb = blobA_sb[0:P, 192:192 + 4 * P]
            v2_sb = blobA_sb[:, 448:448 + GB]
            ones_sb = blobA_sb[:, 450:451]
            hw_sb = blobA_sb[:, 451:451 + BL]
            wihcol_sb = blobA_sb[P:2 * P, 0:4]
            dec_sb = blobB_sb[:, 0:BL * T]
            wdec_sb = blobB_sb[:, BL * T:BL * T + 4 * P]
            wih_sb = blobB_sb[0:1, 768:768 + 4 * P]
            dec_re = dec_sb.rearrange("p (b t) -> p b t", b=BL)

            ht_tiles = [htp[:, b * T:(b + 1) * T] for b in range(BL)]
            HAUG = [haugp[:, b * (M + 1):(b + 1) * (M + 1)] for b in range(BL)]

            # state: 2h and 2c, [64, BL] f16; group g owns cols 2g:2g+2
            SYt = state.tile([P, BL], f16, tag="SYT")
            nc.scalar.dma_start(out=SYt, in_=st0[0:P, :])
            CSt = state.tile([P, BL], f16, tag="CST")
            nc.scalar.dma_start(out=CSt, in_=st0[P:2 * P, :])
            SY = [SYt[:, 2 * g:2 * g + GB] for g in range(NG)]
            CS = [CSt[:, 2 * g:2 * g + GB] for g in range(NG)]

            # UH per group: [(j,m), t] PSUM-resident
            UH = []
            for g in range(NG):
                uh = uh_pool.tile([2 * M, T], f32, tag=f"UH{g}")
                for j in range(GB):
                    nc.tensor.matmul(
                        uh[j * M:(j + 1) * M, :], udT_sb,
                        ht_tiles[g * GB + j], start=True, stop=True,
                    )
                UH.append(uh)

            # ---------- per-step pieces (emitted in pipelined order) ----
            def frontA_PE(g, t):
                """qp matmuls (+ gate-base matmuls unless final attend)."""
                gps = None
                if t is not None:
                    # one accumulation group spans the whole gps bank:
                    # opened by the first whh matmul, closed by the last
                    # wdec matmul (the y~ contribution enters via the Act
                    # bias operand in tail(), not a matmul).
                    gps = loop_ps.tile([P, 4 * GB], f32, tag=f"G{g}")
                    for k in range(4):
                        nc.tensor.matmul(
                            gps[:, k * GB:(k + 1) * GB],
                            whh_sb[:, k * P:(k + 1) * P], SY[g],
                            start=(k == 0), stop=False)
                        nc.tensor.matmul(
                            gps[:, k * GB:(k + 1) * GB],
                            wdec_sb[:, k * P:(k + 1) * P],
                            dec_re[:, 2 * g:2 * g + GB, t],
                            start=False, stop=(k == 3))
                    # gate bases to SBUF (off-chain, 0-cost column copies) so
                    # the gate tanh can take them via its SBUF bias operand.
                    gsb = loop_sb.tile([P, 4 * GB], f32, tag=f"GS{g}")
                    for c in range(4 * GB):
                        nc.vector.tensor_copy(out=gsb[:, c:c + 1],
                                              in_=gps[:, c:c + 1])
                    gps = gsb
                # per-step psum scratch (one full bank): qp | lg | dn | ctx
                sc = loop_ps.tile([2 * P, 512], f32, tag=f"SC{g}")
                qp = sc[:, 0:1]
                # j-chains strictly sequential: concurrent psum groups at
                # different partition offsets trip the zero-region tracker.
                for j in range(GB):
                    nc.tensor.matmul(
                        qp[j * P:(j + 1) * P, :], wdT2_sb[:, M:2 * M],
                        CS[g][:, j:j + 1], start=True, stop=False)
                    nc.tensor.matmul(
                        qp[j * P:(j + 1) * P, :], wdT2_sb[:, 0:M],
                        SY[g][:, j:j + 1], start=False, stop=True)
                return gps, sc

            def frontA_Act(g, sc):
                """bias staging copy (b2b) + the wide attention tanh."""
                bias = loop_sb.tile([2 * P, 1], f32, tag=f"BI{g}")
                nc.scalar.copy(out=bias, in_=sc[:, 0:1])
                e16 = loop_sb.tile([2 * M, T], f16, tag=f"E{g}")
                nc.scalar.activation(out=e16, in_=UH[g], func=AF.Tanh,
                                     bias=bias[:, 0:1], scale=1.0)
                return e16

            def mid_lg(g, sc, e16):
                nc.tensor.matmul(sc[:, 2:2 + GB], e16, v2_sb,
                                 start=True, stop=True)

            def mid_exp(g, sc):
                s16 = loop_sb.tile([T, GB], f16, tag=f"S{g}")
                for j in range(GB):
                    nc.scalar.activation(out=s16[:, j:j + 1],
                                         in_=sc[:, 2 + j:3 + j], func=AF.Exp)
                return s16

            def mid_dn(g, sc, s16, need_num=True):
                # den/num REPLICATED across the 64 gate partitions via
                # broadcast-lhsT matmuls, so the downstream division and
                # wih*y~ products are all single-column (0-cost) DVE ops.
                if not need_num:
                    dn = sc[0:1, 4:4 + 2 * GB]
                    nc.tensor.matmul(dn[0:1, 0:GB], ones_sb, s16,
                                     start=True, stop=True)
                    return dn
                dn64 = sc[0:M, 12:12 + 2 * GB]
                nc.tensor.matmul(
                    dn64[:, 0:GB], ones_sb.to_broadcast([2 * M, M]), s16,
                    start=True, stop=True)
                for j in range(GB):
                    b = 2 * g + j
                    nc.tensor.matmul(
                        dn64[:, GB + j:GB + j + 1],
                        hw_sb[:, b:b + 1].to_broadcast([2 * M, M]),
                        s16[:, j:j + 1], start=True, stop=True)
                return dn64

            def mid_dve(g, sc):
                # wn_kj = wih_k * num_j / den_j  (the W_ih @ y~ softmax part;
                # the w0*dec part is already in gps).  No float divide in the
                # DVE ISA: reciprocal + 2 multiplies, all [64,1] psum-direct.
                dn64 = sc[0:M, 12:12 + 2 * GB]
                rd = loop_sb.tile([P, GB], f32, tag=f"RD{g}")
                numr = loop_sb.tile([P, GB], f32, tag=f"NR{g}")
                for j in range(GB):
                    nc.vector.reciprocal(out=rd[:, j:j + 1],
                                         in_=dn64[:, j:j + 1])
                    nc.vector.tensor_tensor(
                        out=numr[:, j:j + 1], in0=dn64[:, GB + j:GB + j + 1],
                        in1=rd[:, j:j + 1], op=OP.mult)
                return numr

            def tail(g, gps, numr):
                """LSTM state update, all on Act as [P,1] fused FMA ops.
                gate cols i,f,o,g; state is 2h/2c (sigmoid folded to tanh,
                0.5s packed in the weights).  scale operands must be f32.
                Gate pre-activation = wih_k*(num_j/den_j) + base, computed
                entirely inside the tanh's scale/bias operands."""
                tg = loop_sb.tile([P, 4 * GB], f32, tag=f"TG{g}")
                uv = loop_sb.tile([P, 3 * GB], f32, tag=f"UV{g}")
                for j in range(GB):
                    sl = [tg[:, k * GB + j:k * GB + j + 1] for k in range(4)]
                    tgi, tgf, tgo, tgg = sl
                    for k in range(4):
                        nc.scalar.activation(
                            out=sl[k], in_=wihcol_sb[:, k:k + 1],
                            func=AF.Tanh, scale=numr[:, j:j + 1],
                            bias=gps[:, k * GB + j:k * GB + j + 1])
                    u = uv[:, j:j + 1]
                    v = uv[:, GB + j:GB + j + 1]
                    th = uv[:, 2 * GB + j:2 * GB + j + 1]
                    cs = CS[g][:, j:j + 1]
                    sy = SY[g][:, j:j + 1]
                    # u = 2c*(tf+1);  v = tg*(ti+1)
                    nc.scalar.activation(out=u, in_=cs, func=AF.Identity,
                                         scale=tgf, bias=cs)
                    nc.scalar.activation(out=v, in_=tgg, func=AF.Identity,
                                         scale=tgi, bias=tgg)
                    # 2c' = 0.5*u + v ; th = tanh(c') ; 2h' = th*(to+1)
                    nc.scalar.activation(out=cs, in_=u, func=AF.Identity,
                                         scale=0.5, bias=v)
                    nc.scalar.activation(out=th, in_=cs, func=AF.Tanh,
                                         scale=0.5)
                    nc.scalar.activation(out=sy, in_=th, func=AF.Identity,
                                         scale=tgo, bias=th)

            # ---------------- main recurrence (pipelined) ----------------
            # The tile scheduler re-orders by (readiness, priority); left
            # alone it collapses the two chains into lockstep (both 292ns
            # attention tanhs back-to-back on Act, serializing the period to
            # ~1387ns).  The "phase anchor" below — a 0-cost Act read of
            # chain 1's qp psum column right after chain 0's attention tanh —
            # WAR-forces qp1(t) to dispatch after e0(t), pinning the chains
            # ~40% of a period apart in both the scheduling pass and the
            # final schedule; the stagger is then self-sustaining.
            cur0 = frontA_PE(0, 0)
            prev1 = None
            for t in range(T - 1):
                gps0, sc0 = cur0
                e0 = frontA_Act(0, sc0)
                if prev1 is not None:
                    tail(1, *prev1)
                mid_lg(0, sc0, e0)
                s0 = mid_exp(0, sc0)
                gps1, sc1 = frontA_PE(1, t)
                mid_dn(0, sc0, s0)
                wn0 = mid_dve(0, sc0)
                e1 = frontA_Act(1, sc1)
                mid_lg(1, sc1, e1)
                tail(0, gps0, wn0)
                cur0 = frontA_PE(0, t + 1 if t + 1 < T - 1 else None)
                s1 = mid_exp(1, sc1)
                mid_dn(1, sc1, s1)
                wn1 = mid_dve(1, sc1)
                prev1 = (gps1, wn1)
            tail(1, *prev1)

            # ---------------- final attend + outputs ----------------
            ctx_out = state.tile([M + 1, BL], f32, tag="CTXOUT")
            sc_f = {0: cur0[1]}
            for g in range(NG):
                if g == 1:
                    sc_f[1] = frontA_PE(1, None)[1]
                sc = sc_f[g]
                e16 = frontA_Act(g, sc)
                mid_lg(g, sc, e16)
                s16 = mid_exp(g, sc)
                dn = mid_dn(g, sc, s16, need_num=False)
                s_fr = loop_sb.tile([T, GB], f32r, tag=f"SF{g}")
                nc.vector.tensor_copy(out=s_fr, in_=s16)
                ctxp = sc[0:M, 8:8 + 2 * GB]
                for j in range(GB):
                    b = 2 * g + j
                    mm(ctxp[:, 2 * j:2 * j + 2], HAUG[b][:, 0:M],
                       s_fr[:, j:j + 1].to_broadcast([T, 2]),
                       start=True, stop=True)
                nc.vector.tensor_copy(
                    out=ctx_out[0:M, 2 * g:2 * g + GB],
                    in_=ctxp.rearrange("p (j two) -> p j two", two=2)[:, :, 0])
                nc.scalar.copy(out=ctx_out[M:M + 1, 2 * g:2 * g + GB],
                               in_=dn[0:1, 0:GB])
                nc.sync.dma_start(out=oh[:, 2 * g:2 * g + GB], in_=SY[g])
            nc.sync.dma_start(out=octx[:], in_=ctx_out)

    nc.finalize()
    return nc


def _pack_weights(W_d, U_d, v_d, w_tilde_W, w_tilde_b, W_ih, W_hh, b_ih, b_hh):
    f = np.float32
    # q = [h;c] stored as 2h;2c -> fold 0.5 into W_d^T halves
    wdT2 = np.ascontiguousarray(
        0.5 * W_d.T.reshape(2, P, M).transpose(1, 0, 2).reshape(P, 2 * M),
        dtype=np.float16)
    udT = np.ascontiguousarray(U_d.T, dtype=np.float16)         # [64, 64]
    w0 = f(w_tilde_W[0, 0])
    bsum = (b_ih + b_hh).astype(f)
    wih = W_ih[:, 0].astype(f)
    # torch gate order i,f,g,o; our column order i,f,o,g.
    # sigmoid gates (i,f,o): pre-scale 0.5 (sigmoid(z) = 0.5 tanh(0.5 z)+0.5)
    # h input is 2h -> extra 0.5 on W_hh blocks.
    src = [0, 1, 3, 2]                    # i, f, o, g row-blocks in torch order
    sig = [0.5, 0.5, 0.5, 1.0]
    whh64 = np.zeros((P, 4 * P), dtype=np.float16)
    wdec2 = np.zeros((2, 4 * P), dtype=np.float16)
    for k in range(4):
        blk = slice(src[k] * P, (src[k] + 1) * P)
        whh64[:, k * P:(k + 1) * P] = sig[k] * 0.5 * W_hh[blk].T
        wdec2[0, k * P:(k + 1) * P] = sig[k] * bsum[blk]
        wdec2[1, k * P:(k + 1) * P] = sig[k] * wih[blk]
    # blobA: wdT2 | udT | whh64 on rows 0:64; wihcol rows 64:128 cols 0:4;
    # v2 | ones on all rows
    blobA = np.zeros((2 * M, 455), dtype=np.float16)
    blobA[0:P, 0:2 * M] = wdT2
    blobA[0:M, 2 * M:2 * M + M] = udT
    blobA[0:P, 192:192 + 4 * P] = whh64
    for k in range(4):
        blk = slice(src[k] * P, (src[k] + 1) * P)
        blobA[P:2 * P, k] = sig[k] * wih[blk]
    blobA[0:M, 448] = v_d[0]
    blobA[M:2 * M, 449] = v_d[0]
    blobA[:, 450] = 1.0
    return blobA, wdec2, w0, w_tilde_W[0, 1:M + 1], f(w_tilde_b[0])


OUT_NAMES = ["oh", "octx"]


def _core_in_map(wpack, H, dec_data, d_1, s_1, core):
    blobA, wdec2, w0, wt1, wtb = wpack
    sl = slice(core * BL, (core + 1) * BL)
    h_l = np.ascontiguousarray(H[sl])                       # [BL, T, M]
    ht_l = h_l.transpose(0, 2, 1).astype(np.float16)        # [BL, M, T]
    htp = np.ascontiguousarray(
        ht_l.transpose(1, 0, 2).reshape(M, BL * T))
    hw = h_l @ wt1.astype(np.float32) + wtb                 # [BL, T]
    haug_l = np.concatenate([h_l, hw[:, :, None]], axis=2).astype(np.float32)
    haugp = np.ascontiguousarray(
        haug_l.transpose(1, 0, 2).reshape(T, BL * (M + 1)))
    bA = blobA.copy()
    bA[:, 451:451 + BL] = hw.T.astype(np.float16)           # hw16 [T, BL]
    blobB = np.ones((2, 1024), dtype=np.float16)
    blobB[1, 0:BL * T] = (w0 * dec_data[sl, :, 0]).reshape(BL * T)
    blobB[:, BL * T:BL * T + 4 * P] = wdec2
    blobB[0, 768:768 + 4 * P] = wdec2[1]
    st = np.concatenate(
        [2.0 * d_1[0, sl].T, 2.0 * s_1[0, sl].T], axis=0
    ).astype(np.float16)                       # [128, 4]
    return dict(blobA=bA, blobB=blobB, htp=htp, haugp=haugp,
                st0=np.ascontiguousarray(st))


def _unpack_outputs(r):
    shard = np.zeros((BL, 1, P + M), dtype=np.float32)
    hv = r["oh"].astype(np.float32).T * 0.5   # [4, 64]  (state was 2h)
    octx = r["octx"]
    ctx = (octx[0:M] / octx[M:M + 1]).T       # [4, 64]
    shard[:, 0, 0:P] = hv
    shard[:, 0, P:P + M] = ctx
    return shard


def kernel(H, dec_data, d_1, s_1, W_d, U_d, v_d, w_tilde_W, w_tilde_b,
           W_ih, W_hh, b_ih, b_hh, T=None):
    from concourse.bass_utils import run_bass_kernel_spmd

    H = np.asarray(H, dtype=np.float32)
    dec_data = np.asarray(dec_data, dtype=np.float32)
    d_1 = np.asarray(d_1, dtype=np.float32)
    s_1 = np.asarray(s_1, dtype=np.float32)

    if "nc" not in _STATE:
        _STATE["nc"] = _build_nc()
    nc = _STATE["nc"]

    wpack = _pack_weights(
        np.asarray(W_d, np.float32), np.asarray(U_d, np.float32),
        np.asarray(v_d, np.float32), np.asarray(w_tilde_W, np.float32),
        np.asarray(w_tilde_b, np.float32), np.asarray(W_ih, np.float32),
        np.asarray(W_hh, np.float32), np.asarray(b_ih, np.float32),
        np.asarray(b_hh, np.float32),
    )

    in_maps = [
        _core_in_map(wpack, H, dec_data, d_1, s_1, core)
        for core in range(NCORES)
    ]

    res = run_bass_kernel_spmd(nc, in_maps, list(range(NCORES)))
    _STATE["last_results"] = res

    out = np.zeros((B, 1, P + M), dtype=np.float32)
    for core in range(NCORES):
        out[core * BL:(core + 1) * BL] = _unpack_outputs(res.results[core])
    return out
